# revision 1
# baseline (speedup 1.0000x reference)
"""Trainium2 Bass kernel for nn_ControlFlowExpert_62380105007397.

Reference semantics (CPU-XLA eager jax):
  x: [16, 8192, 208] fp32.
  imm = sequential fp32 chain sum_n x[..., 195+n] * 16^n   (n = 0..7)
  pc  = same over cols 171..178
  ax  = int32-wrap sum of trunc-toward-zero casts of cols 163..170 times 16^n
  any_jmp/any_bz/any_bnz = global any() of opcode cols 90/92/93 > 0.5
  If any flag set: out = x with cols 171..178 = nibbles of int32(new_pc)
  and col 203 = branch-taken flag; else out = x.

Strategy: flags are computed on host (3 column scans) and select a
compile-time specialized device kernel. The dominant any_jmp path runs
fully on device: stream x through SBUF in 1.7MB tiles on 8 cores (batch
sharded), compute imm with the exact fp32 chain order (DVE is IEEE fp32,
bit-identical to XLA CPU), truncate toward zero with an RNE-cast +
correction (HW cast rounds to nearest), extract nibbles with arithmetic
shifts, splice in place, stream out. Rare paths (bz/bnz without jmp) use
a host-computed 9-column patch spliced on device while streaming.
"""

import sys

if "/opt/trn_rl_repo" not in sys.path:
    sys.path.insert(0, "/opt/trn_rl_repo")

import numpy as np

B, T, C = 16, 8192, 208
N_CORES = 8
ROWS_PER_CORE = (B // N_CORES) * T          # 16384
P = 128                                     # SBUF partitions
W = 16                                      # rows per partition per tile
TILE_ROWS = P * W                           # 2048
N_TILES = ROWS_PER_CORE // TILE_ROWS        # 8

OPC_JMP, OPC_BZ, OPC_BNZ = 90, 92, 93
AX0, PC0, IMM0, BT = 163, 171, 195, 203

_kernel_cache = {}

# perf knobs (test harness overrides these before first kernel() call)
CONFIG = {"W": 16, "out_engine": "scalar", "csplit": 1, "bufs": 4}


def _emit_compute(nc, mybir, sp, x3, tag):
    """DVE pipeline on one [P, ws] row-slice view x3 of the x tile."""
    A = mybir.AluOpType
    f32, i32 = mybir.dt.float32, mybir.dt.int32
    ws = x3.shape[1]

    # imm = ((x195*1 + x196*16) + x197*256) ... sequential fp32 chain
    acc = sp.tile([P, ws], f32, tag=f"acc0{tag}")
    nc.vector.scalar_tensor_tensor(
        out=acc[:], in0=x3[:, :, IMM0 + 1], scalar=16.0,
        in1=x3[:, :, IMM0], op0=A.mult, op1=A.add)
    for n in range(2, 8):
        nacc = sp.tile([P, ws], f32, tag=f"acc{n}{tag}")
        nc.vector.scalar_tensor_tensor(
            out=nacc[:], in0=x3[:, :, IMM0 + n], scalar=float(16.0 ** n),
            in1=acc[:], op0=A.mult, op1=A.add)
        acc = nacc

    # trunc toward zero: y = rne_cast(acc); d = acc - f(y);
    # correction fires when RNE moved away from zero.
    y = sp.tile([P, ws], i32, tag=f"y{tag}")
    nc.vector.tensor_copy(out=y[:], in_=acc[:])
    fy = sp.tile([P, ws], f32, tag=f"fy{tag}")
    nc.vector.tensor_copy(out=fy[:], in_=y[:])
    d = sp.tile([P, ws], f32, tag=f"d{tag}")
    nc.vector.scalar_tensor_tensor(
        out=d[:], in0=fy[:], scalar=-1.0, in1=acc[:], op0=A.mult, op1=A.add)
    a1 = sp.tile([P, ws], f32, tag=f"a1{tag}")
    nc.vector.tensor_scalar(out=a1[:], in0=d[:], scalar1=0.0, scalar2=None,
                            op0=A.is_lt)
    m1 = sp.tile([P, ws], f32, tag=f"m1{tag}")
    nc.vector.scalar_tensor_tensor(
        out=m1[:], in0=acc[:], scalar=0.0, in1=a1[:], op0=A.is_gt, op1=A.mult)
    a2 = sp.tile([P, ws], f32, tag=f"a2{tag}")
    nc.vector.tensor_scalar(out=a2[:], in0=d[:], scalar1=0.0, scalar2=None,
                            op0=A.is_gt)
    m2 = sp.tile([P, ws], f32, tag=f"m2{tag}")
    nc.vector.scalar_tensor_tensor(
        out=m2[:], in0=acc[:], scalar=0.0, in1=a2[:], op0=A.is_lt, op1=A.mult)
    ft = sp.tile([P, ws], f32, tag=f"ft{tag}")
    nc.vector.scalar_tensor_tensor(
        out=ft[:], in0=m1[:], scalar=-1.0, in1=fy[:], op0=A.mult, op1=A.add)
    ft2 = sp.tile([P, ws], f32, tag=f"ft2{tag}")
    nc.vector.tensor_add(out=ft2[:], in0=ft[:], in1=m2[:])
    v = sp.tile([P, ws], i32, tag=f"v{tag}")
    nc.vector.tensor_copy(out=v[:], in_=ft2[:])

    # nibbles: sh[n] = v >> 4n; nib[n] = sh[n] - 16*sh[n+1]
    sh = [v]
    for n in range(1, 9):
        s = sp.tile([P, ws], i32, tag=f"s{n}{tag}")
        nc.vector.tensor_scalar(
            out=s[:], in0=v[:] if n <= 7 else sh[7][:],
            scalar1=4 * n if n <= 7 else 4, scalar2=None,
            op0=A.arith_shift_right)
        sh.append(s)
    for n in range(8):
        nc.vector.scalar_tensor_tensor(
            out=x3[:, :, PC0 + n], in0=sh[n + 1][:], scalar=-16.0,
            in1=sh[n][:], op0=A.mult, op1=A.add)
    nc.vector.memset(x3[:, :, BT], 1.0)


def _build_jmp_kernel():
    """Device kernel for the any_jmp path: everything on device."""
    import concourse.bacc as bacc
    import concourse.mybir as mybir
    from concourse.tile import TileContext

    f32 = mybir.dt.float32
    W = CONFIG["W"]
    csplit = CONFIG["csplit"]
    tile_rows = P * W
    n_tiles = ROWS_PER_CORE // tile_rows

    nc = bacc.Bacc("TRN2")
    out_eng = getattr(nc, CONFIG["out_engine"])
    x = nc.dram_tensor("x", [ROWS_PER_CORE, C], f32, kind="ExternalInput")
    out = nc.dram_tensor("out", [ROWS_PER_CORE, C], f32, kind="ExternalOutput")

    with TileContext(nc) as tc:
        with tc.tile_pool(name="sbuf", bufs=CONFIG["bufs"]) as pool, \
             tc.tile_pool(name="small", bufs=2) as sp:
            for t in range(n_tiles):
                rows = slice(t * tile_rows, (t + 1) * tile_rows)
                xt = pool.tile([P, W * C], f32, tag="xt")
                x3 = xt[:].rearrange("p (w c) -> p w c", c=C)
                nc.sync.dma_start(
                    out=xt[:],
                    in_=x[rows, :].rearrange("(p w) c -> p (w c)", p=P))
                ws = W // csplit
                out2 = out[rows, :].rearrange("(p w) c -> p (w c)", p=P)
                for h in range(csplit):
                    _emit_compute(nc, mybir, sp, x3[:, h * ws:(h + 1) * ws, :],
                                  tag=f"h{h}")
                    out_eng.dma_start(
                        out=out2[:, h * ws * C:(h + 1) * ws * C],
                        in_=xt[:, h * ws * C:(h + 1) * ws * C])
    nc.finalize()
    return nc


def _emit_compute_raw(nc, mybir, tmp, x3):
    """DVE pipeline on one [P, ws] row-slice view x3, raw-bass variant.
    tmp: dict of preallocated scratch SBUF tensors. Returns last instr.
    nc.vector.drain() between RAW-dependent DVE ops — raw bass does not get
    the automatic per-op drains Tile inserts, and the DVE pipe otherwise
    lets op N+1 read SBUF before op N's write has committed."""
    A = mybir.AluOpType
    dr = nc.vector.drain
    acc_cur, acc_nxt = tmp["accA"], tmp["accB"]
    nc.vector.scalar_tensor_tensor(
        out=acc_cur[:], in0=x3[:, :, IMM0 + 1], scalar=16.0,
        in1=x3[:, :, IMM0], op0=A.mult, op1=A.add)
    for n in range(2, 8):
        dr()
        nc.vector.scalar_tensor_tensor(
            out=acc_nxt[:], in0=x3[:, :, IMM0 + n], scalar=float(16.0 ** n),
            in1=acc_cur[:], op0=A.mult, op1=A.add)
        acc_cur, acc_nxt = acc_nxt, acc_cur
    acc = acc_cur
    y, fy, d = tmp["y"], tmp["fy"], tmp["d"]
    a1, m1, a2, m2, ft, ft2, v = (tmp[k] for k in
                                  ("a1", "m1", "a2", "m2", "ft", "ft2", "v"))
    dr()
    nc.vector.tensor_copy(out=y[:], in_=acc[:])
    dr()
    nc.vector.tensor_copy(out=fy[:], in_=y[:])
    dr()
    nc.vector.scalar_tensor_tensor(
        out=d[:], in0=fy[:], scalar=-1.0, in1=acc[:], op0=A.mult, op1=A.add)
    dr()
    nc.vector.tensor_scalar(out=a1[:], in0=d[:], scalar1=0.0, scalar2=None,
                            op0=A.is_lt)
    dr()
    nc.vector.scalar_tensor_tensor(
        out=m1[:], in0=acc[:], scalar=0.0, in1=a1[:], op0=A.is_gt, op1=A.mult)
    nc.vector.tensor_scalar(out=a2[:], in0=d[:], scalar1=0.0, scalar2=None,
                            op0=A.is_gt)
    dr()
    nc.vector.scalar_tensor_tensor(
        out=m2[:], in0=acc[:], scalar=0.0, in1=a2[:], op0=A.is_lt, op1=A.mult)
    dr()
    nc.vector.scalar_tensor_tensor(
        out=ft[:], in0=m1[:], scalar=-1.0, in1=fy[:], op0=A.mult, op1=A.add)
    dr()
    nc.vector.tensor_add(out=ft2[:], in0=ft[:], in1=m2[:])
    dr()
    nc.vector.tensor_copy(out=v[:], in_=ft2[:])
    dr()
    sh = [v]
    for n in range(1, 8):
        s = tmp[f"s{n}"]
        nc.vector.tensor_scalar(out=s[:], in0=v[:], scalar1=4 * n,
                                scalar2=None, op0=A.arith_shift_right)
        sh.append(s)
    dr()
    s8 = tmp["s8"]
    nc.vector.tensor_scalar(out=s8[:], in0=sh[7][:], scalar1=4, scalar2=None,
                            op0=A.arith_shift_right)
    sh.append(s8)
    dr()
    for n in range(8):
        nc.vector.scalar_tensor_tensor(
            out=x3[:, :, PC0 + n], in0=sh[n + 1][:], scalar=-16.0,
            in1=sh[n][:], op0=A.mult, op1=A.add)
    nc.vector.memset(x3[:, :, BT], 1.0)
    return dr()


def _build_jmp_raw():
    """Raw-bass (no TileContext) pipelined jmp kernel: minimal fixed cost."""
    from contextlib import ExitStack

    import concourse.bacc as bacc
    import concourse.mybir as mybir

    f32, i32 = mybir.dt.float32, mybir.dt.int32
    W = CONFIG["W"]
    csplit = CONFIG["csplit"]
    ws = W // csplit
    tile_rows = P * W
    T = ROWS_PER_CORE // tile_rows

    nc = bacc.Bacc("TRN2")
    x = nc.dram_tensor("x", [ROWS_PER_CORE, C], f32, kind="ExternalInput")
    out = nc.dram_tensor("out", [ROWS_PER_CORE, C], f32, kind="ExternalOutput")

    with ExitStack() as st:
        slots = [st.enter_context(nc.sbuf_tensor(f"xs{t}", [P, W * C], f32))
                 for t in range(T)]
        tmp = {}
        for k in ("accA", "accB", "fy", "d", "a1", "m1", "a2", "m2",
                  "ft", "ft2"):
            tmp[k] = st.enter_context(nc.sbuf_tensor(f"t_{k}", [P, ws], f32))
        for k in ("y", "v", "s1", "s2", "s3", "s4", "s5", "s6", "s7", "s8"):
            tmp[k] = st.enter_context(nc.sbuf_tensor(f"t_{k}", [P, ws], i32))
        sem_in = [st.enter_context(nc.semaphore(f"sin{t}")) for t in range(T)]
        sem_cmp = st.enter_context(nc.semaphore("scmp"))
        sem_out = st.enter_context(nc.semaphore("sout"))
        block = st.enter_context(nc.Block())

        pace = CONFIG.get("pace", 0)

        @block.sync
        def _(sync):
            for t in range(T):
                if pace and t >= pace:
                    # keep IN issuance ~pace tiles ahead of compute so the
                    # out-ring interleaves instead of backlogging at the end
                    sync.wait_ge(sem_cmp, csplit * (t - pace + 1))
                rows = slice(t * tile_rows, (t + 1) * tile_rows)
                sync.dma_start(
                    slots[t][:],
                    x[rows, :].rearrange("(p w) c -> p (w c)", p=P),
                ).then_inc(sem_in[t], 16)

        @block.vector
        def _(vector):
            for t in range(T):
                vector.wait_ge(sem_in[t], 16)
                x3 = slots[t][:].rearrange("p (w c) -> p w c", c=C)
                for h in range(csplit):
                    last = _emit_compute_raw(
                        nc, mybir, tmp, x3[:, h * ws:(h + 1) * ws, :])
                    last.then_inc(sem_cmp, 1)

        @block.scalar
        def _(scalar):
            for t in range(T):
                rows = slice(t * tile_rows, (t + 1) * tile_rows)
                out2 = out[rows, :].rearrange("(p w) c -> p (w c)", p=P)
                for h in range(csplit):
                    scalar.wait_ge(sem_cmp, t * csplit + h + 1)
                    scalar.dma_start(
                        out2[:, h * ws * C:(h + 1) * ws * C],
                        slots[t][:, h * ws * C:(h + 1) * ws * C],
                    ).then_inc(sem_out, 16)
            scalar.wait_ge(sem_out, 16 * csplit * T)

    nc.finalize()
    return nc


def _build_patch_kernel():
    """Device kernel for rare flag combos: stream x, splice host patch."""
    import concourse.bacc as bacc
    import concourse.mybir as mybir
    from concourse.tile import TileContext

    f32 = mybir.dt.float32
    nc = bacc.Bacc("TRN2")
    x = nc.dram_tensor("x", [ROWS_PER_CORE, C], f32, kind="ExternalInput")
    patch = nc.dram_tensor("patch", [ROWS_PER_CORE, 9], f32, kind="ExternalInput")
    out = nc.dram_tensor("out", [ROWS_PER_CORE, C], f32, kind="ExternalOutput")

    with TileContext(nc) as tc:
        with tc.tile_pool(name="sbuf", bufs=4) as pool, \
             tc.tile_pool(name="small", bufs=3) as sp:
            for t in range(N_TILES):
                rows = slice(t * TILE_ROWS, (t + 1) * TILE_ROWS)
                xt = pool.tile([P, W * C], f32, tag="xt")
                x3 = xt[:].rearrange("p (w c) -> p w c", c=C)
                nc.sync.dma_start(
                    out=xt[:],
                    in_=x[rows, :].rearrange("(p w) c -> p (w c)", p=P))
                pt = sp.tile([P, W * 9], f32, tag="pt")
                p3 = pt[:].rearrange("p (w c) -> p w c", c=9)
                nc.sync.dma_start(
                    out=pt[:],
                    in_=patch[rows, :].rearrange("(p w) c -> p (w c)", p=P))
                nc.vector.tensor_copy(out=x3[:, :, PC0:PC0 + 8], in_=p3[:, :, 0:8])
                nc.vector.tensor_copy(out=x3[:, :, BT], in_=p3[:, :, 8])
                nc.sync.dma_start(
                    out=out[rows, :].rearrange("(p w) c -> p (w c)", p=P),
                    in_=xt[:])
    nc.finalize()
    return nc


def _get_kernel(name):
    if name not in _kernel_cache:
        if name == "jmp":
            builder = _build_jmp_raw if CONFIG.get("raw") else _build_jmp_kernel
            _kernel_cache[name] = builder()
        else:
            _kernel_cache[name] = _build_patch_kernel()
    return _kernel_cache[name]


# test.py can set _RUN_KWARGS["trace"] = True and read LAST for profiling.
_RUN_KWARGS = {}
LAST = None


def _run_spmd(nc, in_maps):
    global LAST
    from concourse.bass_utils import run_bass_kernel_spmd
    LAST = run_bass_kernel_spmd(nc, in_maps, core_ids=list(range(N_CORES)),
                                **_RUN_KWARGS)
    return LAST


def _host_patch(x):
    """Exact CPU-XLA-equivalent computation of the 9 modified columns."""
    pw = np.float32(16.0) ** np.arange(8, dtype=np.float32)
    imm = x[..., IMM0].astype(np.float32)
    pc = x[..., PC0].astype(np.float32)
    for n in range(1, 8):
        imm = (x[..., IMM0 + n] * pw[n] + imm).astype(np.float32)
        pc = (x[..., PC0 + n] * pw[n] + pc).astype(np.float32)
    axs = np.zeros(x.shape[:-1], dtype=np.int64)
    for n in range(8):
        axs += x[..., AX0 + n].astype(np.int32).astype(np.int64) * (16 ** n)
    ax = ((axs + 2**31) % 2**32 - 2**31).astype(np.int32)
    ax_is_zero = ax == 0

    any_jmp = bool((x[..., OPC_JMP] > 0.5).any())
    any_bz = bool((x[..., OPC_BZ] > 0.5).any())
    any_bnz = bool((x[..., OPC_BNZ] > 0.5).any())

    pc8 = (pc + np.float32(8.0)).astype(np.float32)
    if any_jmp:
        new_pc = imm
        bt = np.ones_like(imm)
    elif any_bz:
        new_pc = np.where(ax_is_zero, imm, pc8)
        bt = ax_is_zero.astype(np.float32)
    else:  # any_bnz
        new_pc = np.where(~ax_is_zero, imm, pc8)
        bt = (~ax_is_zero).astype(np.float32)
    v = new_pc.astype(np.int32)
    shifts = np.arange(8, dtype=np.int32) * 4
    nibs = ((v[..., None] >> shifts) & 15).astype(np.float32)
    return np.concatenate([nibs, bt[..., None]], axis=-1)


def kernel(x):
    x = np.ascontiguousarray(np.asarray(x), dtype=np.float32)
    assert x.shape == (B, T, C), x.shape

    any_jmp = bool((x[..., OPC_JMP] > 0.5).any())
    any_bz = bool((x[..., OPC_BZ] > 0.5).any())
    any_bnz = bool((x[..., OPC_BNZ] > 0.5).any())
    if not (any_jmp or any_bz or any_bnz):
        return x.copy()

    xf = x.reshape(N_CORES, ROWS_PER_CORE, C)
    if any_jmp:
        nc = _get_kernel("jmp")
        in_maps = [{"x": xf[c]} for c in range(N_CORES)]
    else:
        nc = _get_kernel("patch")
        patch = _host_patch(x).reshape(N_CORES, ROWS_PER_CORE, 9)
        in_maps = [{"x": xf[c], "patch": patch[c]} for c in range(N_CORES)]

    res = _run_spmd(nc, in_maps)
    out = np.empty((N_CORES, ROWS_PER_CORE, C), dtype=np.float32)
    for c in range(N_CORES):
        out[c] = res.results[c]["out"]
    return out.reshape(B, T, C)



# revision 2
# speedup vs baseline: 4.3431x; 4.3431x over previous
"""Trainium2 Bass kernel for nn_ControlFlowExpert_62380105007397.

Reference semantics (CPU-XLA eager jax):
  x: [16, 8192, 208] fp32.
  imm = sequential fp32 chain sum_n x[..., 195+n] * 16^n   (n = 0..7)
  pc  = same over cols 171..178
  ax  = int32-wrap sum of trunc-toward-zero casts of cols 163..170 times 16^n
  any_jmp/any_bz/any_bnz = global any() of opcode cols 90/92/93 > 0.5
  If any flag set: out = x with cols 171..178 = nibbles of int32(new_pc)
  and col 203 = branch-taken flag; else out = x.

Strategy: the op reads only 8 input columns and modifies only 9 output
columns; everything else is identity. Flags are computed on host (3
column scans) and select a compile-time specialized device kernel. The
dominant any_jmp path: host slices the 8 imm columns into a compact
[rows, 8] fp32 array per core (batch-sharded over 8 cores); the device
computes the exact sequential fp32 chain (bit-identical to XLA CPU),
truncates toward zero (RNE cast + sign-bit-or correction, HW-verified
exact), and extracts all 8 nibbles with 4 uint16-bitcast shift-and ops,
writing compact nibble planes. Host assembles out = x.copy() with the
device nibbles spliced into cols 171..178 and col 203 = 1. Device HBM
traffic drops 37x vs streaming all 208 columns. Rare paths (bz/bnz
without jmp) use a host-computed 9-column patch spliced on device.
"""

import sys

if "/opt/trn_rl_repo" not in sys.path:
    sys.path.insert(0, "/opt/trn_rl_repo")

import numpy as np

B, T, C = 16, 8192, 208
N_CORES = 8
ROWS_PER_CORE = (B // N_CORES) * T          # 16384
P = 128                                     # SBUF partitions
W = ROWS_PER_CORE // P                      # 128 rows per partition

OPC_JMP, OPC_BZ, OPC_BNZ = 90, 92, 93
AX0, PC0, IMM0, BT = 163, 171, 195, 203

_kernel_cache = {}

# perf knobs (test harness overrides these before first kernel() call)
CONFIG = {"tiles": 2, "bufs": 2, "out_engine": "scalar"}


def _emit_compact(nc, mybir, pool, sp, a3, nibt, tag):
    """DVE pipeline: a3 [P, Wt, 8] fp32 view -> nibt [P, 4, 2*Wt] u16 view.

    chain (7 STT, exact sequential fp32 order) -> trunc toward zero
    (9 ops, HW-verified) -> 4 shift-and nibble ops on the u16 bitcast.
    """
    A = mybir.AluOpType
    f32, i32, u32, u16 = (mybir.dt.float32, mybir.dt.int32, mybir.dt.uint32,
                          mybir.dt.uint16)
    ws = a3.shape[1]

    acc = sp.tile([P, ws], f32, tag=f"acc{tag}")
    nc.vector.scalar_tensor_tensor(
        out=acc[:], in0=a3[:, :, 1], scalar=16.0,
        in1=a3[:, :, 0], op0=A.mult, op1=A.add)
    for n in range(2, 8):
        nacc = sp.tile([P, ws], f32, tag=f"acc{n}{tag}")
        nc.vector.scalar_tensor_tensor(
            out=nacc[:], in0=a3[:, :, n], scalar=float(16.0 ** n),
            in1=acc[:], op0=A.mult, op1=A.add)
        acc = nacc

    # trunc toward zero: y = rne(acc); fy = f(y); d = acc - fy; w = d*fy;
    # corr = or(bits(w<0), signbit(fy)); ft = fy - corr; v = i32(ft)
    y = sp.tile([P, ws], i32, tag=f"y{tag}")
    nc.vector.tensor_copy(out=y[:], in_=acc[:])
    fy = sp.tile([P, ws], f32, tag=f"fy{tag}")
    nc.vector.tensor_copy(out=fy[:], in_=y[:])
    d = sp.tile([P, ws], f32, tag=f"d{tag}")
    nc.vector.scalar_tensor_tensor(
        out=d[:], in0=fy[:], scalar=-1.0, in1=acc[:], op0=A.mult, op1=A.add)
    w = sp.tile([P, ws], f32, tag=f"w{tag}")
    nc.vector.tensor_tensor(out=w[:], in0=d[:], in1=fy[:], op=A.mult)
    mask = sp.tile([P, ws], f32, tag=f"mask{tag}")
    nc.vector.tensor_scalar(out=mask[:], in0=w[:], scalar1=0.0, scalar2=None,
                            op0=A.is_lt)
    sb = sp.tile([P, ws], u32, tag=f"sb{tag}")
    nc.vector.tensor_scalar(out=sb[:], in0=fy[:].bitcast(u32),
                            scalar1=0x80000000, scalar2=None,
                            op0=A.bitwise_and)
    corr = sp.tile([P, ws], u32, tag=f"corr{tag}")
    nc.vector.tensor_tensor(out=corr[:], in0=mask[:].bitcast(u32),
                            in1=sb[:], op=A.bitwise_or)
    ft = sp.tile([P, ws], f32, tag=f"ft{tag}")
    nc.vector.tensor_tensor(out=ft[:], in0=fy[:], in1=corr[:].bitcast(f32),
                            op=A.subtract)
    v = sp.tile([P, ws], i32, tag=f"v{tag}")
    nc.vector.tensor_copy(out=v[:], in_=ft[:])

    # nibbles: u16 view [P, 2*ws]: elem (2w+j) = halfword j of row w.
    # plane n holds nibble n (j=0) and nibble n+4 (j=1) of each row.
    v16 = v[:].bitcast(u16)
    nc.vector.tensor_scalar(out=nibt[:, 0, :], in0=v16, scalar1=15,
                            scalar2=None, op0=A.bitwise_and)
    for n in range(1, 4):
        nc.vector.tensor_scalar(out=nibt[:, n, :], in0=v16,
                                scalar1=4 * n, scalar2=15,
                                op0=A.logical_shift_right,
                                op1=A.bitwise_and)


def _build_jmp_compact():
    """any_jmp path: compact columns in, nibble planes out."""
    import concourse.bacc as bacc
    import concourse.mybir as mybir
    from concourse.tile import TileContext

    f32 = mybir.dt.float32
    u16 = mybir.dt.uint16
    nt = CONFIG["tiles"]
    wt = W // nt                    # rows per partition per tile
    tile_rows = P * wt

    nc = bacc.Bacc("TRN2")
    out_eng = getattr(nc, CONFIG["out_engine"])
    a = nc.dram_tensor("a", [ROWS_PER_CORE, 8], f32, kind="ExternalInput")
    onib = nc.dram_tensor("onib", [nt, P, 8 * wt], u16, kind="ExternalOutput")

    with TileContext(nc) as tc:
        with tc.tile_pool(name="sbuf", bufs=CONFIG["bufs"]) as pool, \
             tc.tile_pool(name="small", bufs=2) as sp:
            for t in range(nt):
                rows = slice(t * tile_rows, (t + 1) * tile_rows)
                at = pool.tile([P, wt * 8], f32, tag="at")
                nc.sync.dma_start(
                    out=at[:],
                    in_=a[rows, :].rearrange("(p w) c -> p (w c)", p=P))
                a3 = at[:].rearrange("p (w c) -> p w c", c=8)
                nibt = pool.tile([P, 8 * wt], u16, tag="nibt")
                n3 = nibt[:].rearrange("p (n w) -> p n w", n=4)
                _emit_compact(nc, mybir, pool, sp, a3, n3, tag="")
                out_eng.dma_start(out=onib[t], in_=nibt[:])
    nc.finalize()
    return nc


def _build_patch_kernel():
    """Device kernel for rare flag combos: stream x, splice host patch."""
    import concourse.bacc as bacc
    import concourse.mybir as mybir
    from concourse.tile import TileContext

    f32 = mybir.dt.float32
    W16 = 16
    TILE_ROWS = P * W16
    N_TILES = ROWS_PER_CORE // TILE_ROWS

    nc = bacc.Bacc("TRN2")
    x = nc.dram_tensor("x", [ROWS_PER_CORE, C], f32, kind="ExternalInput")
    patch = nc.dram_tensor("patch", [ROWS_PER_CORE, 9], f32, kind="ExternalInput")
    out = nc.dram_tensor("out", [ROWS_PER_CORE, C], f32, kind="ExternalOutput")

    with TileContext(nc) as tc:
        with tc.tile_pool(name="sbuf", bufs=4) as pool, \
             tc.tile_pool(name="small", bufs=3) as sp:
            for t in range(N_TILES):
                rows = slice(t * TILE_ROWS, (t + 1) * TILE_ROWS)
                xt = pool.tile([P, W16 * C], f32, tag="xt")
                x3 = xt[:].rearrange("p (w c) -> p w c", c=C)
                nc.sync.dma_start(
                    out=xt[:],
                    in_=x[rows, :].rearrange("(p w) c -> p (w c)", p=P))
                pt = sp.tile([P, W16 * 9], f32, tag="pt")
                p3 = pt[:].rearrange("p (w c) -> p w c", c=9)
                nc.sync.dma_start(
                    out=pt[:],
                    in_=patch[rows, :].rearrange("(p w) c -> p (w c)", p=P))
                nc.vector.tensor_copy(out=x3[:, :, PC0:PC0 + 8], in_=p3[:, :, 0:8])
                nc.vector.tensor_copy(out=x3[:, :, BT], in_=p3[:, :, 8])
                nc.sync.dma_start(
                    out=out[rows, :].rearrange("(p w) c -> p (w c)", p=P),
                    in_=xt[:])
    nc.finalize()
    return nc


def _get_kernel(name):
    if name not in _kernel_cache:
        if name == "jmp":
            _kernel_cache[name] = _build_jmp_compact()
        else:
            _kernel_cache[name] = _build_patch_kernel()
    return _kernel_cache[name]


# test.py can set _RUN_KWARGS["trace"] = True and read LAST for profiling.
_RUN_KWARGS = {}
LAST = None


def _run_spmd(nc, in_maps):
    global LAST
    from concourse.bass_utils import run_bass_kernel_spmd
    LAST = run_bass_kernel_spmd(nc, in_maps, core_ids=list(range(N_CORES)),
                                **_RUN_KWARGS)
    return LAST


def _host_patch(x):
    """Exact CPU-XLA-equivalent computation of the 9 modified columns."""
    pw = np.float32(16.0) ** np.arange(8, dtype=np.float32)
    imm = x[..., IMM0].astype(np.float32)
    pc = x[..., PC0].astype(np.float32)
    for n in range(1, 8):
        imm = (x[..., IMM0 + n] * pw[n] + imm).astype(np.float32)
        pc = (x[..., PC0 + n] * pw[n] + pc).astype(np.float32)
    axs = np.zeros(x.shape[:-1], dtype=np.int64)
    for n in range(8):
        axs += x[..., AX0 + n].astype(np.int32).astype(np.int64) * (16 ** n)
    ax = ((axs + 2**31) % 2**32 - 2**31).astype(np.int32)
    ax_is_zero = ax == 0

    any_bz = bool((x[..., OPC_BZ] > 0.5).any())

    pc8 = (pc + np.float32(8.0)).astype(np.float32)
    if any_bz:
        new_pc = np.where(ax_is_zero, imm, pc8)
        bt = ax_is_zero.astype(np.float32)
    else:  # any_bnz
        new_pc = np.where(~ax_is_zero, imm, pc8)
        bt = (~ax_is_zero).astype(np.float32)
    v = new_pc.astype(np.int32)
    shifts = np.arange(8, dtype=np.int32) * 4
    nibs = ((v[..., None] >> shifts) & 15).astype(np.float32)
    return np.concatenate([nibs, bt[..., None]], axis=-1)


def kernel(x):
    x = np.ascontiguousarray(np.asarray(x), dtype=np.float32)
    assert x.shape == (B, T, C), x.shape

    any_jmp = bool((x[..., OPC_JMP] > 0.5).any())
    any_bz = bool((x[..., OPC_BZ] > 0.5).any())
    any_bnz = bool((x[..., OPC_BNZ] > 0.5).any())
    if not (any_jmp or any_bz or any_bnz):
        return x.copy()

    xr = x.reshape(N_CORES, ROWS_PER_CORE, C)
    if any_jmp:
        nc = _get_kernel("jmp")
        a = np.ascontiguousarray(xr[:, :, IMM0:IMM0 + 8])
        in_maps = [{"a": a[c]} for c in range(N_CORES)]
        res = _run_spmd(nc, in_maps)

        out = x.copy()
        orows = out.reshape(N_CORES, ROWS_PER_CORE, C)
        nt = CONFIG["tiles"]
        wt = W // nt
        for c in range(N_CORES):
            nib = res.results[c]["onib"]          # [nt, P, 8*wt] u16
            nib = nib.reshape(nt, P, 4, wt, 2)    # [t, p, plane, w, half]
            # row = (t*P + p)*wt + w ; col = 4*half + plane
            nib = nib.transpose(0, 1, 3, 4, 2).reshape(ROWS_PER_CORE, 8)
            orows[c, :, PC0:PC0 + 8] = nib.astype(np.float32)
        orows[:, :, BT] = 1.0
        return out

    nc = _get_kernel("patch")
    patch = _host_patch(x).reshape(N_CORES, ROWS_PER_CORE, 9)
    in_maps = [{"x": xr[c], "patch": patch[c]} for c in range(N_CORES)]
    res = _run_spmd(nc, in_maps)
    out = np.empty((N_CORES, ROWS_PER_CORE, C), dtype=np.float32)
    for c in range(N_CORES):
        out[c] = res.results[c]["out"]
    return out.reshape(B, T, C)
